# revision 30
# baseline (speedup 1.0000x reference)
"""DistributionMaxPool Trainium2 kernel.

Math insight: the reference's CxC conv sums the selected 2x2-strided pixel
over ALL input channels and replicates across output channels. Every
per-channel value after that reduction is identical, so the whole
Gaussian-max pipeline runs on channel-summed planes and the result is
broadcast back across the 128 channels at the end.

Per-core pipeline (batch-parallel over 8 cores, 4 batches each):
  1. DMA each [128, 64*64] plane (contiguous 2 MB) into SBUF (sync/SP
     queue only, so input streaming is never head-of-line blocked).
  2. Channel sums via pixels-stationary matmuls: lhsT = single-stride
     (step 2) view of the plane covering one s-parity of 4 consecutive
     rows (128 pixels), rhs = ones [128, 1], N=1. Each matmul writes one
     PSUM column; 2x16 matmuls per plane fill two [128, 16] PSUM tiles
     (s=0 / s=1) with layout:
       partition p = 64u + 32r + j, col q   ->  pixel (i = 2q + u, j)
     for output pixel (i, j), 2x2 offset (r, s).
  3. Gaussian-max math. DVE does arithmetic (incl. rsqrt via bit-trick +
     3 Newton steps, reciprocal for the exp identity); ACT does only
     erf / tanh / square -- all in ONE activation table
     (sigmoid_and_others), so there is a single table load. exp(-t) =
     (1-tanh(t/2))/(1+tanh(t/2)). Stage B runs as one fused [32, 32]
     call after base-aligning copies (2-src ops need equal bases).
  4. Output: [32, 32] grids vector-transposed, sheared to a flat row by
     SWDGE DMA, partition-broadcast to [128, 1024] on GPSIMD, stored as
     one contiguous 512 KB SWDGE DMA per (batch, plane). The whole
     output path issues from the Pool queue.
"""

import sys

if "/opt/trn_rl_repo" not in sys.path:
    sys.path.insert(0, "/opt/trn_rl_repo")

import numpy as np

B_FULL = 32
N_CORES = 8
B = B_FULL // N_CORES  # 4 batches per core
C = 128
H = W = 64
HO = WO = 32
NPIX = HO * WO  # 1024

EPS = 1e-8
INV_SQRT2 = float(1.0 / np.sqrt(2.0))
INV_SQRT_2PI = float(1.0 / np.sqrt(2.0 * np.pi))
RSQRT_MAGIC = 0x5F3759DF

_CACHE = {}


def _gauss_max(nc, pool, m1, v1, m2, v2, out_mean, out_var, P, F):
    """mean/var of max of two Gaussians, elementwise on [P, F] views.

    d = m1-m2, p = m1+m2, s = v1+v2+eps, rs = 1/sqrt(s), alpha = s*rs,
    beta = d*rs, e = erf(beta/sqrt2), g = exp(-beta^2/2):
      mean = p/2 + e*d/2 + c2*alpha*g
      var  = s/2 + (p^2+d^2)/4 + eps + e*(d*p + (v1-v2))/2 + c2*p*alpha*g
             - mean^2
    ACT ops: erf, tanh, square only (single activation table).
    """
    import concourse.mybir as mybir

    f32 = mybir.dt.float32
    i32 = mybir.dt.int32
    Act = mybir.ActivationFunctionType
    mult = mybir.AluOpType.mult
    add = mybir.AluOpType.add
    shr = mybir.AluOpType.arith_shift_right

    def t(name, dtype=f32):
        return pool.tile([P, F], dtype, name=name, tag=f"{name}_{P}x{F}")

    s_ = t("gs")
    nc.vector.scalar_tensor_tensor(s_[:], v1, EPS, v2, add, add)
    # rs = rsqrt(s): bit-trick seed + 3 Newton iterations (fp32-exact)
    sh = t("gsh", i32)
    nc.vector.tensor_scalar(sh[:], s_[:].bitcast(i32), 1, None, shr)
    yi = t("gy", i32)
    nc.vector.tensor_scalar(yi[:], sh[:], -1, RSQRT_MAGIC, mult, add)
    yf = yi[:].bitcast(f32)
    nt1 = t("gnt1")
    nt2 = t("gnt2")
    for _ in range(3):
        nc.vector.tensor_mul(nt1[:], yf, yf)
        nc.vector.scalar_tensor_tensor(nt2[:], nt1[:], -0.5, s_[:], mult, mult)
        nc.vector.scalar_tensor_tensor(yf, nt2[:], 1.5, yf, add, mult)
    alpha = t("galpha")
    nc.vector.tensor_mul(alpha[:], s_[:], yf)
    d = t("gd")
    nc.vector.tensor_sub(d[:], m1, m2)
    beta = t("gbeta")
    nc.vector.tensor_mul(beta[:], d[:], yf)
    e = t("ge")
    nc.scalar.activation(e[:], beta[:], Act.Erf, scale=INV_SQRT2)
    b2 = t("gb2")
    nc.scalar.square(b2[:], beta[:])
    # g = exp(-b2/2) = (1-T)/(1+T), T = tanh(b2/4)
    T = t("gT")
    nc.scalar.activation(T[:], b2[:], Act.Tanh, scale=0.25)
    num = t("gnum")
    nc.vector.tensor_scalar(num[:], T[:], -1.0, 1.0, mult, add)
    den = t("gden")
    nc.vector.tensor_scalar(den[:], T[:], 1.0, None, add)
    dr = t("gdr")
    nc.vector.reciprocal(dr[:], den[:])
    g = t("gg")
    nc.vector.tensor_mul(g[:], num[:], dr[:])

    p_ = t("gp")
    nc.vector.tensor_add(p_[:], m1, m2)
    ag = t("gag")
    nc.vector.tensor_mul(ag[:], alpha[:], g[:])
    # mean = 0.5*p + 0.5*e*d + c2*ag
    u_ = t("gu")
    nc.vector.scalar_tensor_tensor(u_[:], e[:], 0.5, d[:], mult, mult)
    w_ = t("gw")
    nc.vector.scalar_tensor_tensor(w_[:], p_[:], 0.5, u_[:], mult, add)
    nc.vector.scalar_tensor_tensor(out_mean, ag[:], INV_SQRT_2PI, w_[:], mult, add)
    # var
    dv = t("gdv")
    nc.vector.tensor_sub(dv[:], v1, v2)
    dp = t("gdp")
    nc.vector.tensor_mul(dp[:], d[:], p_[:])
    z = t("gz")
    nc.vector.tensor_add(z[:], dp[:], dv[:])
    ez = t("gez")
    nc.vector.scalar_tensor_tensor(ez[:], e[:], 0.5, z[:], mult, mult)
    d2 = t("gd2")
    nc.scalar.square(d2[:], d[:])
    p2 = t("gp2")
    nc.scalar.square(p2[:], p_[:])
    pd = t("gpd")
    nc.vector.tensor_add(pd[:], p2[:], d2[:])
    qd = t("gqd")
    nc.vector.tensor_scalar(qd[:], pd[:], 0.25, EPS, mult, add)
    acc = t("gacc")
    nc.vector.scalar_tensor_tensor(acc[:], s_[:], 0.5, qd[:], mult, add)
    v3 = t("gv3")
    nc.vector.tensor_add(v3[:], ez[:], acc[:])
    pag = t("gpag")
    nc.vector.tensor_mul(pag[:], p_[:], ag[:])
    v4 = t("gv4")
    nc.vector.scalar_tensor_tensor(v4[:], pag[:], INV_SQRT_2PI, v3[:], mult, add)
    v5 = t("gv5")
    nc.scalar.square(v5[:], out_mean)
    nc.vector.scalar_tensor_tensor(out_var, v5[:], -1.0, v4[:], mult, add)


def _kernel_body(nc, tc, x, y, ones, xin, sums, math_pool, rowp, bcp, psp):
    import concourse.mybir as mybir

    f32 = mybir.dt.float32
    for b in range(B):
        sm = sums.tile([128, 32], f32, name="sm", tag="sm")
        sv = sums.tile([128, 32], f32, name="sv", tag="sv")
        # Variance plane first: the rsqrt/alpha dependency spine needs only
        # variances, so it starts before the mean plane lands.
        for pl in (1, 0):
            xt = xin.tile([C, H * W], f32, name="xt", tag="xt")
            nc.sync.dma_start(xt[:], x[b, pl].rearrange("c h w -> c (h w)"))
            x3 = xt[:].rearrange("c (q m s) -> c q m s", q=16, m=128, s=2)
            dst = sm if pl == 0 else sv
            for s in range(2):
                ps = psp.tile([128, 16], f32, name="ps", tag="ps")
                for q in range(16):
                    nc.tensor.matmul(
                        ps[:, q : q + 1],
                        x3[:, q, :, s],
                        ones[:, 0:1],
                        start=True,
                        stop=True,
                    )
                nc.vector.tensor_copy(dst[:, 16 * s : 16 * s + 16], ps[:])

        # Stage A: s=0 vs s=1 (free-dim split), full 128 partitions.
        hm = math_pool.tile([128, 16], f32, name="hm", tag="hm")
        hv = math_pool.tile([128, 16], f32, name="hv", tag="hv")
        _gauss_max(
            nc, math_pool,
            sm[:, 0:16], sv[:, 0:16], sm[:, 16:32], sv[:, 16:32],
            hm[:], hv[:], 128, 16,
        )
        # Stage B: r=0 vs r=1 inside each u half. Base-align all four
        # operands into [32, 32] tiles with copies (2-src ops need equal
        # input base partitions), cols 0:16 = u0, 16:32 = u1.
        m1c = math_pool.tile([32, 32], f32, name="m1c", tag="m1c")
        v1c = math_pool.tile([32, 32], f32, name="v1c", tag="v1c")
        m2c = math_pool.tile([32, 32], f32, name="m2c", tag="m2c")
        v2c = math_pool.tile([32, 32], f32, name="v2c", tag="v2c")
        for u in range(2):
            base = 64 * u
            cols = slice(16 * u, 16 * u + 16)
            nc.vector.tensor_copy(m1c[:, cols], hm[base : base + 32, :])
            nc.vector.tensor_copy(v1c[:, cols], hv[base : base + 32, :])
            nc.vector.tensor_copy(m2c[:, cols], hm[base + 32 : base + 64, :])
            nc.vector.tensor_copy(v2c[:, cols], hv[base + 32 : base + 64, :])
        # Two stage-B calls (one per u half) writing through single-stride
        # views so memory layout is col = i = 2q+u (one contiguous shear
        # DMA per plane). Single-stride DVE destinations are HW-verified;
        # multi-dim strided destinations are not.
        meant = math_pool.tile([32, 32], f32, name="meant", tag="meant")
        vart = math_pool.tile([32, 32], f32, name="vart", tag="vart")
        meant3 = meant[:].rearrange("p (q u) -> p q u", u=2)
        vart3 = vart[:].rearrange("p (q u) -> p q u", u=2)
        for u in range(2):
            cols = slice(16 * u, 16 * u + 16)
            _gauss_max(
                nc, math_pool,
                m1c[:, cols], v1c[:, cols], m2c[:, cols], v2c[:, cols],
                meant3[:, :, u], vart3[:, :, u], 32, 16,
            )

        # Output per plane: transpose (memory layout is already col = i);
        # one contiguous shear DMA to a flat row (scalar HWDGE queue);
        # GPSIMD partition broadcast; one contiguous 512 KB store (Pool
        # queue). The mean plane's store overlaps the var plane's math.
        for pl, src in ((0, meant), (1, vart)):
            mt = math_pool.tile([32, 32], f32, name="mt", tag="mt")
            nc.vector.transpose(mt[:], src[:])
            row = rowp.tile([1, NPIX], f32, name="row", tag="row")
            nc.scalar.dma_start(row[:], mt[:])
            bc = bcp.tile([128, NPIX], f32, name="bcst", tag="bcst")
            nc.gpsimd.partition_broadcast(bc[:], row[:])
            # Last batch: store from the idle SP queue (all inputs already
            # issued, so no head-of-line risk); earlier batches via Pool.
            out_eng = nc.sync if b == B - 1 else nc.gpsimd
            out_eng.dma_start(y[b, pl].rearrange("c h w -> c (h w)"), bc[:])


def _build(reps=1, timing=False):
    import concourse.bacc as bacc
    import concourse.mybir as mybir
    import concourse.tile as tile

    f32 = mybir.dt.float32
    nc = bacc.Bacc("TRN2", target_bir_lowering=False, debug=False, num_devices=N_CORES)

    x = nc.declare_dram_parameter("x", [B, 2, C, H, W], f32, isOutput=False)
    if timing:
        # Device work identical, but keep the big output internal so the
        # relay doesn't download 32 MB per timed call. A tiny external
        # output reads y back so none of the stores are dead.
        y = nc.dram_tensor("y_int", [B, 2, C, HO, WO], f32)
        y_small = nc.declare_dram_parameter("ysum", [1, 4], f32, isOutput=True)
    else:
        y = nc.declare_dram_parameter("y", [B, 2, C, HO, WO], f32, isOutput=True)

    with tile.TileContext(nc) as tc:
        with (
            tc.tile_pool(name="xin", bufs=3) as xin,
            tc.tile_pool(name="const", bufs=1) as const,
            tc.tile_pool(name="sums", bufs=2) as sums,
            tc.tile_pool(name="math", bufs=3) as math_pool,
            tc.tile_pool(name="row", bufs=4) as rowp,
            tc.tile_pool(name="bc", bufs=3) as bcp,
            tc.tile_pool(name="ps", bufs=3, space="PSUM") as psp,
        ):
            ones = const.tile([128, 1], f32)
            nc.gpsimd.memset(ones[:], 1.0)

            for _rep in range(reps):
                _kernel_body(nc, tc, x, y, ones, xin, sums, math_pool, rowp, bcp, psp)

            if timing:
                rb = rowp.tile([1, 4], f32, name="rb", tag="rb")
                nc.sync.dma_start(rb[:], y[0, 0].rearrange("c h w -> c (h w)")[0:1, 0:4])
                nc.sync.dma_start(y_small[:], rb[:])

    nc.compile()
    return nc


def _get_nc():
    if "nc" not in _CACHE:
        _CACHE["nc"] = _build()
    return _CACHE["nc"]


def kernel(x: np.ndarray) -> np.ndarray:
    from concourse.bass_utils import run_bass_kernel_spmd

    assert x.shape == (B_FULL, 2, C, H, W), x.shape
    x = np.ascontiguousarray(x, dtype=np.float32)
    nc = _get_nc()
    in_maps = [{"x": x[i * B : (i + 1) * B]} for i in range(N_CORES)]
    res = run_bass_kernel_spmd(nc, in_maps, list(range(N_CORES)))
    return np.concatenate([res.results[i]["y"] for i in range(N_CORES)], axis=0)
